# revision 32
# baseline (speedup 1.0000x reference)
"""Trainium2 Bass kernel for nn_DiagonalVariance: per-dim MLPs [4->64->64->1] with softplus.

Strategy (pure data parallel over batch, 8 cores):
  - Host packs x^T = [y^T; t^T] as [19, B] fp16 so all device DMAs are contiguous.
  - Per dim-pair p (2 dims), weights are packed as:
      W1p [19, 128]  (y-rows are delta-masked per dim, t-rows shared); b1 via ACT bias
      W2p [128, 128] block-diagonal of two 64x64 blocks
      W3p [128, 16]  cols 2p/2p+1 hold W3 for the two dims, rest zero
  - softplus computed two ways, balancing the Scalar (ACT) and Vector (DVE) engines:
      A-path (ACT): softplus(z) = Ln(Exp(z) + 1), Exp fused with bias, E in fp16.
      D-path (DVE): lhsT pre-halved so psum holds s = z/2; then
        softplus(z) = s + s^2*R(s^2) + ln2 with R a deg-4 minimax poly;
        evaluated as fp16 TSP/TT ops (ln2 constant folded into next layer bias).
  - All SBUF tensors fp16 (PE full rate); psum fp32. NB=1024 so both z pools
    double-buffer within 8 psum banks (2 banks/tile), removing PE<->consumer
    serialization; z3 shares the z2 pool so the next tile's L1 can start early.
  - Output written as [16, BC] per core, transposed on the host.
"""

import numpy as np
from contextlib import ExitStack, nullcontext

import concourse.bass as bass
import concourse.bacc as bacc
import concourse.tile as tile
from concourse import mybir
from concourse.hw_specs import get_activation_tables

F = mybir.ActivationFunctionType
ALU = mybir.AluOpType
FP32 = mybir.dt.float32
FP16 = mybir.dt.float16

B = 262144
D = 16
TE = 3
H = 64
NCORES = 8
BC = B // NCORES          # 32768 batch points per core
NB = 1024                 # batch tile
NMM = 512                 # psum bank: max fp32 free dim per matmul
NPAIR = D // 2            # 8 dim-pairs
NTILES = BC // NB

_ACT_SET = "natural_log_exp_and_others"

# minimax fit of (ln(2cosh(s)) - ln2)/s^2 in m = s^2 on m in [0, 12.25]
# (covers |z| <= 7; max abs err of the softplus 1.15e-2)
RPOLY = (0.47135357903606767, -0.04741247795339884,
         0.003631342144629101, -0.00011271774634781165)
LN2 = float(np.log(2.0))

# which (layer, pair) slots run the DVE polynomial path (tuned for balance);
# ALT_SLOTS run the DVE path on odd tiles only (halved weights live in an
# extra packed block, ln2 added in-chain since shared biases can't alternate)
DVE_SLOTS = frozenset({(0, 3), (1, 3), (0, 7), (1, 7)})
ALT_SLOTS = ()
# slots whose polynomial chain runs on the GPSIMD (Pool) engine: the psum
# extract and square stay on DVE (GPSIMD cannot read psum), the SBUF-only
# Horner steps run on Pool
POOL_SLOTS = frozenset({(1, 6)})


def _pin_act_tables(arch):
    """Restrict Exp/Ln to one table set so bacc emits a single table load."""
    tables = get_activation_tables(arch)
    for name, funcs in tables.items():
        if name != _ACT_SET:
            funcs.discard(F.Exp)
            funcs.discard(F.Ln)


def build(ntiles=NTILES, reps=1, nb=NB, ln_group=4, dve_slots=DVE_SLOTS,
          alt_slots=ALT_SLOTS, pool_slots=POOL_SLOTS, num_devices=NCORES):
    nc = bacc.Bacc("TRN2", target_bir_lowering=False, debug=False,
                   enable_asserts=True, num_devices=num_devices)
    _pin_act_tables(nc.m.arch)
    NB = nb
    G = ln_group

    # tiles batch in groups of `ob`: the final softplus runs once per batch on
    # an accumulator packing 4 tiles per [128, NB] block at partition offsets
    # 0/32/64/96 (dims in rows 32a..32a+15; rows 32a+16.. are memset garbage)
    ob = next(o for o in (8, 4, 2, 1) if ntiles % o == 0)
    ncolblk = max(1, ob // 4)

    xT = nc.dram_tensor("xT", [D + TE, BC], FP16, kind="ExternalInput")
    w1 = nc.dram_tensor("w1", [D + TE, NPAIR * 128], FP16, kind="ExternalInput")
    nw2 = NPAIR + sum(1 for s in alt_slots if s[0] == 1)
    w2 = nc.dram_tensor("w2", [128, nw2 * 128], FP16, kind="ExternalInput")
    w3 = nc.dram_tensor("w3", [128, NPAIR * 16], FP16, kind="ExternalInput")
    b1 = nc.dram_tensor("b1", [128, NPAIR], FP32, kind="ExternalInput")
    b2 = nc.dram_tensor("b2", [128, nw2], FP32, kind="ExternalInput")
    b3 = nc.dram_tensor("b3", [128, 1], FP32, kind="ExternalInput")
    # output row 32*(i%4)+d, col (i//4)*NB+j holds dim d of point i*NB+j
    nblk = (ntiles + 3) // 4
    out = nc.dram_tensor("out", [128, nblk * NB], FP16, kind="ExternalOutput")

    mm = nc.tensor.matmul

    def softplus_dve(vec, z, beta_ap, s_dst, u_dst, add_ln2=False, chain=None):
        """softplus(z+beta) = s + u (+ln2 folded downstream): s = (z+beta)/1,
        u = s^2*R(s^2).  s and u feed the next matmul as separate accumulating
        rhs streams, so no final tensor-tensor add is needed.  The psum
        extract and square always run on DVE; `chain` (default DVE) runs the
        SBUF-only Horner steps (nc.gpsimd offloads them to Pool)."""
        ch = chain or vec
        vec.tensor_scalar(s_dst, z, beta_ap, None, ALU.add)
        m = vec.pool.tile([128, NB], FP16, tag="m")
        vec.tensor_tensor(m, s_dst, s_dst, ALU.mult)
        r = vec.pool.tile([128, NB], FP16, tag="r")
        c = RPOLY
        ch.tensor_scalar(r, m, float(c[3]), float(c[2]), ALU.mult, ALU.add)
        for k in (1, 0):
            ch.tensor_tensor(r, r, m, ALU.mult)
            ch.tensor_scalar(r, r, float(c[k]), None, ALU.add)
        ch.tensor_tensor(u_dst, r, m, ALU.mult)
        if add_ln2:
            ch.tensor_scalar(u_dst, u_dst, LN2, None, ALU.add)

    with tile.TileContext(nc) as tc:
        with ExitStack() as ctx:
            wpool = ctx.enter_context(tc.tile_pool(name="w", bufs=1))
            xpool = ctx.enter_context(tc.tile_pool(name="x", bufs=2))
            hpool1 = ctx.enter_context(tc.tile_pool(name="h1", bufs=3))
            hpool2 = ctx.enter_context(tc.tile_pool(name="h2", bufs=2))
            apool = ctx.enter_context(tc.tile_pool(name="acc", bufs=1))
            opool = ctx.enter_context(tc.tile_pool(name="o", bufs=1))
            epool = ctx.enter_context(tc.tile_pool(name="e", bufs=3))
            e3pool = ctx.enter_context(tc.tile_pool(name="e3", bufs=1))
            vpool = ctx.enter_context(tc.tile_pool(name="v", bufs=2))
            zpool1 = ctx.enter_context(tc.tile_pool(name="z1", bufs=2, space="PSUM"))
            zpool2 = ctx.enter_context(tc.tile_pool(name="z2", bufs=2, space="PSUM"))

            class _Vec:
                pool = vpool
                tensor_scalar = nc.vector.tensor_scalar
                tensor_tensor = nc.vector.tensor_tensor
            vec = _Vec()

            w1sb = wpool.tile([D + TE, NPAIR * 128], FP16)
            w2sb = wpool.tile([128, nw2 * 128], FP16)
            w3sb = wpool.tile([128, NPAIR * 16], FP16)
            b1sb = wpool.tile([128, NPAIR], FP32)
            b2sb = wpool.tile([128, nw2], FP32)
            b3sb = wpool.tile([128, 1], FP32)
            nc.sync.dma_start(out=w1sb, in_=w1[:, :])
            nc.sync.dma_start(out=w2sb, in_=w2[:, :])
            nc.sync.dma_start(out=w3sb, in_=w3[:, :])
            nc.sync.dma_start(out=b1sb, in_=b1[:, :])
            nc.sync.dma_start(out=b2sb, in_=b2[:, :])
            nc.sync.dma_start(out=b3sb, in_=b3[:, :])

            ngroup = NPAIR // G

            def emit_l1(i):
                """DMA x tile, L1 matmuls + softplus. Returns h1src per group."""
                xt = xpool.tile([D + TE, NB], FP16)
                nc.sync.dma_start(out=xt, in_=xT[:, i * NB:(i + 1) * NB])
                h1srcs = []
                for g in range(ngroup):
                    pg = list(range(g * G, (g + 1) * G))
                    ea = [j for j, p in enumerate(pg)
                          if (0, p) not in dve_slots and (0, p) not in pool_slots]
                    e1g = epool.tile([128, G, NB], FP16, tag="e")
                    h1g = hpool1.tile([128, G, NB], FP16)
                    h1src = []
                    for j, p in enumerate(pg):
                        z1 = zpool1.tile([128, NB], FP32, tag="z1")
                        for q in range(NB // NMM):
                            s_ = slice(q * NMM, (q + 1) * NMM)
                            mm(z1[:, s_], w1sb[:, p * 128:(p + 1) * 128],
                               xt[:, s_], start=True, stop=True)
                        if (0, p) in dve_slots or (0, p) in pool_slots:
                            sd = hpool1.tile([128, NB], FP16, tag="s1")
                            ud = hpool1.tile([128, NB], FP16, tag="hd1")
                            h1src.append((sd, ud))
                            softplus_dve(vec, z1, b1sb[:, p:p + 1], sd, ud,
                                         chain=nc.gpsimd
                                         if (0, p) in pool_slots else None)
                        else:
                            h1src.append((h1g[:, j, :],))
                            nc.scalar.activation(e1g[:, j, :], z1, F.Exp,
                                                 bias=b1sb[:, p:p + 1])
                    if ea:
                        # contiguous runs only; assignment keeps A-slots contiguous
                        j0, j1 = ea[0], ea[-1] + 1
                        nc.scalar.activation(h1g[:, j0:j1, :], e1g[:, j0:j1, :],
                                             F.Ln, bias=1.0)
                    h1srcs.append(h1src)
                return h1srcs

            alt2 = {s: NPAIR + n for n, s in
                    enumerate(s for s in alt_slots if s[0] == 1)}

            def emit_l2(i, h1srcs):
                """L2 matmuls + softplus. Returns h2src per group."""
                odd = i % 2 == 1
                h2srcs = []
                for g in range(ngroup):
                    pg = list(range(g * G, (g + 1) * G))
                    h1src = h1srcs[g]
                    dve2 = [p for p in pg if (1, p) in dve_slots
                            or (1, p) in pool_slots or (odd and (1, p) in alt2)]
                    ea2 = [j for j, p in enumerate(pg) if p not in dve2]
                    e2g = epool.tile([128, G, NB], FP16, tag="e")
                    h2g = hpool2.tile([128, G, NB], FP16)
                    h2src = []
                    for j, p in enumerate(pg):
                        is_alt = odd and (1, p) in alt2
                        pb = alt2[(1, p)] if is_alt else p
                        z2 = zpool2.tile([128, NB], FP32, tag="z2")
                        srcs = h1src[j]
                        for q in range(NB // NMM):
                            s_ = slice(q * NMM, (q + 1) * NMM)
                            for si, src in enumerate(srcs):
                                mm(z2[:, s_], w2sb[:, pb * 128:(pb + 1) * 128],
                                   src[:, s_], start=(si == 0),
                                   stop=(si == len(srcs) - 1))
                        if p in dve2:
                            sd = hpool2.tile([128, NB], FP16, tag="s2")
                            ud = hpool2.tile([128, NB], FP16, tag="hd2")
                            h2src.append((sd, ud))
                            softplus_dve(vec, z2, b2sb[:, pb:pb + 1], sd, ud,
                                         add_ln2=is_alt,
                                         chain=nc.gpsimd
                                         if (1, p) in pool_slots else None)
                        else:
                            h2src.append((h2g[:, j, :],))
                            nc.scalar.activation(e2g[:, j, :], z2, F.Exp,
                                                 bias=b2sb[:, pb:pb + 1])
                    if ea2:
                        j0, j1 = ea2[0], ea2[-1] + 1
                        nc.scalar.activation(h2g[:, j0:j1, :], e2g[:, j0:j1, :],
                                             F.Ln, bias=1.0)
                    h2srcs.append(h2src)
                return h2srcs

            def emit_l3(i, h2srcs, acc):
                """L3 psum accumulation + fold into acc; final softplus per batch."""
                u, q3 = i % ob, i // ob
                arow = acc[32 * (u % 4):32 * (u % 4) + D,
                           (u // 4) * NB:(u // 4) * NB + NB]
                z3 = zpool2.tile([16, NB], FP32, tag="z2")
                for g in range(ngroup):
                    pg = list(range(g * G, (g + 1) * G))
                    h2src = h2srcs[g]
                    last_g = g == ngroup - 1
                    for q in range(NB // NMM):
                        s_ = slice(q * NMM, (q + 1) * NMM)
                        for j, p in enumerate(pg):
                            for si, src in enumerate(h2src[j]):
                                mm(z3[:, s_], w3sb[:, p * 16:(p + 1) * 16],
                                   src[:, s_],
                                   start=(g == 0 and j == 0 and si == 0),
                                   stop=(last_g and j == G - 1
                                         and si == len(h2src[j]) - 1))
                nc.vector.tensor_copy(arow, z3)
                if u == ob - 1:
                    e3 = e3pool.tile([128, ncolblk * NB], FP16)
                    nc.scalar.activation(e3, acc, F.Exp, bias=b3sb)
                    o3 = opool.tile([128, ncolblk * NB], FP16)
                    nc.scalar.activation(o3, e3, F.Ln, bias=1.0)
                    nc.sync.dma_start(
                        out=out[:, q3 * ncolblk * NB:(q3 + 1) * ncolblk * NB],
                        in_=o3)

            loop_cm = tc.For_i(0, reps, 1) if reps > 1 else nullcontext()
            with loop_cm:
                # software pipeline: L1 of tile i+1 is emitted before L3 of
                # tile i so the in-order PE queue never stalls on the DVE
                # chains feeding tile i's L3
                acc = None
                h1s = emit_l1(0)
                for i in range(ntiles):
                    if i % ob == 0:
                        acc = apool.tile([128, ncolblk * NB], FP32)
                        nc.gpsimd.memset(acc, 0.0)
                    h2s = emit_l2(i, h1s)
                    if i + 1 < ntiles:
                        h1n = emit_l1(i + 1)
                    emit_l3(i, h2s, acc)
                    if i + 1 < ntiles:
                        h1s = h1n
    nc.compile()
    return nc


def _pack_inputs(t, y, W1, b1, W2, b2, W3, b3, dve_slots=DVE_SLOTS,
                 alt_slots=ALT_SLOTS, pool_slots=POOL_SLOTS):
    dve_slots = frozenset(dve_slots) | frozenset(pool_slots)
    """Host-side packing. Returns per-core input maps."""
    t = np.asarray(t, np.float32)
    y = np.asarray(y, np.float32)
    W1 = np.asarray(W1, np.float32)
    b1 = np.asarray(b1, np.float32)
    W2 = np.asarray(W2, np.float32)
    b2 = np.asarray(b2, np.float32)
    W3 = np.asarray(W3, np.float32)
    b3 = np.asarray(b3, np.float32)

    xT = np.empty((D + TE, B), np.float16)
    xT[:D] = y.T
    xT[D:D + TE] = t.T

    # ln2 constants dropped by the D-path fold forward into the next bias.
    l1_dve = np.array([(0, p) in dve_slots for p in range(NPAIR)])
    b2eff = b2 + np.where(
        np.repeat(l1_dve, 2)[:, None], LN2 * W2.sum(axis=1), 0.0)
    l2_dve = np.array([(1, p) in dve_slots for p in range(NPAIR)])
    b3eff = b3 + np.where(
        np.repeat(l2_dve, 2)[:, None], LN2 * W3.sum(axis=1), 0.0)

    alt2 = [s[1] for s in alt_slots if s[0] == 1]
    nw2 = NPAIR + len(alt2)
    w1p = np.zeros((D + TE, NPAIR * 128), np.float16)
    w2p = np.zeros((128, nw2 * 128), np.float16)
    w3p = np.zeros((128, NPAIR * 16), np.float16)
    b1p = np.zeros((128, NPAIR), np.float32)
    b2p = np.zeros((128, nw2), np.float32)
    b3p = np.zeros((128, 1), np.float32)
    for p in range(NPAIR):
        s1 = 0.5 if (0, p) in dve_slots else 1.0
        s2 = 0.5 if (1, p) in dve_slots else 1.0
        for a in range(2):
            d = 2 * p + a
            c = slice(p * 128 + 64 * a, p * 128 + 64 * a + 64)
            w1p[d, c] = s1 * W1[d, 0, :]
            w1p[D:D + TE, c] = s1 * W1[d, 1:1 + TE, :]
            w2p[64 * a:64 * a + 64, p * 128 + 64 * a:p * 128 + 64 * a + 64] = \
                s2 * W2[d]
            w3p[64 * a:64 * a + 64, p * 16 + d] = W3[d, :, 0]
            b1p[64 * a:64 * a + 64, p] = s1 * b1[d]
            b2p[64 * a:64 * a + 64, p] = s2 * b2eff[d]
            for a_ in range(4):
                b3p[32 * a_ + d, 0] = b3eff[d, 0]
    # alt blocks: halved W2/b2 for odd-tile DVE path (ln2 added in-chain)
    for n_, p in enumerate(alt2):
        pb = NPAIR + n_
        for a in range(2):
            d = 2 * p + a
            w2p[64 * a:64 * a + 64, pb * 128 + 64 * a:pb * 128 + 64 * a + 64] = \
                0.5 * W2[d]
            b2p[64 * a:64 * a + 64, pb] = 0.5 * b2eff[d]

    in_maps = []
    for c in range(NCORES):
        in_maps.append({
            "xT": np.ascontiguousarray(xT[:, c * BC:(c + 1) * BC]),
            "w1": w1p, "w2": w2p, "w3": w3p,
            "b1": b1p, "b2": b2p, "b3": b3p,
        })
    return in_maps


def _unpack_output(results):
    cores = []
    for c in range(NCORES):
        a = results[c]["out"]          # [128, nblk*NB]
        nblk = a.shape[1] // NB
        a = (a.reshape(4, 32, nblk, NB)[:, :D]
             .transpose(1, 2, 0, 3).reshape(D, nblk * 4 * NB))
        cores.append(a[:, :BC].T.astype(np.float32))
    return np.concatenate(cores, axis=0)


def make_runner(nc):
    """Build a reusable jitted SPMD callable for `nc` (axon PJRT path)."""
    import jax
    from jax.sharding import Mesh, PartitionSpec, NamedSharding
    from jax.experimental.shard_map import shard_map
    from concourse import bass2jax

    bass2jax.install_neuronx_cc_hook()
    partition_name = nc.partition_id_tensor.name if nc.partition_id_tensor else None
    in_names, out_names, out_avals = [], [], []
    for alloc in nc.m.functions[0].allocations:
        if not isinstance(alloc, mybir.MemoryLocationSet):
            continue
        name = alloc.memorylocations[0].name
        if alloc.kind == "ExternalInput":
            if name != partition_name:
                in_names.append(name)
        elif alloc.kind == "ExternalOutput":
            out_names.append(name)
            out_avals.append(jax.core.ShapedArray(tuple(alloc.tensor_shape),
                                                  mybir.dt.np(alloc.dtype)))
    all_in = in_names + out_names + ([partition_name] if partition_name else [])

    def _body(*args):
        operands = list(args)
        if partition_name is not None:
            operands.append(bass2jax.partition_id_tensor())
        outs = bass2jax._bass_exec_p.bind(
            *operands, out_avals=tuple(out_avals),
            in_names=tuple(all_in), out_names=tuple(out_names),
            lowering_input_output_aliases=(), sim_require_finite=True,
            sim_require_nnan=True, nc=nc)
        return tuple(outs)

    mesh = Mesh(np.asarray(jax.devices()[:NCORES]), ("core",))
    n = len(in_names) + len(out_names)
    sharded = jax.jit(shard_map(_body, mesh=mesh,
                                in_specs=(PartitionSpec("core"),) * n,
                                out_specs=(PartitionSpec("core"),) * len(out_names),
                                check_rep=False), keep_unused=True)
    shard0 = NamedSharding(mesh, PartitionSpec("core"))
    zeros = [jax.device_put(np.zeros((NCORES * a.shape[0], *a.shape[1:]), a.dtype),
                            shard0) for a in out_avals]

    def stage(in_maps):
        return [jax.device_put(
            np.concatenate([np.asarray(in_maps[c][nm]) for c in range(NCORES)], axis=0),
            shard0) for nm in in_names]

    def run_staged(dev_in):
        out_arrs = sharded(*dev_in, *zeros)
        jax.block_until_ready(out_arrs)
        return out_arrs

    def run(in_maps):
        out_arrs = run_staged(stage(in_maps))
        return [
            {name: np.asarray(out_arrs[i]).reshape(NCORES, *out_avals[i].shape)[c]
             for i, name in enumerate(out_names)}
            for c in range(NCORES)
        ]

    run.stage = stage
    run.run_staged = run_staged
    run.out_names = out_names
    run.out_avals = out_avals
    return run


_CACHED = {}


def _get_runner():
    if "runner" not in _CACHED:
        _CACHED["runner"] = make_runner(build())
    return _CACHED["runner"]


def kernel(t, y, W1, b1, W2, b2, W3, b3):
    run = _get_runner()
    in_maps = _pack_inputs(t, y, W1, b1, W2, b2, W3, b3)
    results = run(in_maps)
    return _unpack_output(results)


# revision 33
# speedup vs baseline: 2.2341x; 2.2341x over previous
"""Trainium2 Bass kernel for nn_DiagonalVariance: per-dim MLPs [4->64->64->1] with softplus.

Strategy (pure data parallel over batch, 8 cores):
  - Host packs x^T = [y^T; t^T] as [19, B] fp16 so all device DMAs are contiguous.
  - Per dim-pair p (2 dims), weights are packed as:
      W1p [19, 128]  (y-rows are delta-masked per dim, t-rows shared); b1 via ACT bias
      W2p [128, 128] block-diagonal of two 64x64 blocks
      W3p [128, 16]  cols 2p/2p+1 hold W3 for the two dims, rest zero
  - softplus computed two ways, balancing the Scalar (ACT) and Vector (DVE) engines:
      A-path (ACT): softplus(z) = Ln(Exp(z) + 1), Exp fused with bias, E in fp16.
      D-path (DVE): lhsT pre-halved so psum holds s = z/2; then
        softplus(z) = s + s^2*R(s^2) + ln2 with R a deg-4 minimax poly;
        evaluated as fp16 TSP/TT ops (ln2 constant folded into next layer bias).
  - All SBUF tensors fp16 (PE full rate); psum fp32. NB=1024 so both z pools
    double-buffer within 8 psum banks (2 banks/tile), removing PE<->consumer
    serialization; z3 shares the z2 pool so the next tile's L1 can start early.
  - Output written as [16, BC] per core, transposed on the host.
"""

import numpy as np
from contextlib import ExitStack, nullcontext

import concourse.bass as bass
import concourse.bacc as bacc
import concourse.tile as tile
from concourse import mybir
from concourse.hw_specs import get_activation_tables

F = mybir.ActivationFunctionType
ALU = mybir.AluOpType
FP32 = mybir.dt.float32
FP16 = mybir.dt.float16

B = 262144
D = 16
TE = 3
H = 64
NCORES = 8
BC = B // NCORES          # 32768 batch points per core
NB = 1024                 # batch tile
NMM = 512                 # psum bank: max fp32 free dim per matmul
NPAIR = D // 2            # 8 dim-pairs
NTILES = BC // NB

_ACT_SET = "natural_log_exp_and_others"

# minimax fit of (ln(2cosh(s)) - ln2)/s^2 in m = s^2 on m in [0, 12.25]
# (covers |z| <= 7; max abs err of the softplus 1.15e-2)
RPOLY = (0.47135357903606767, -0.04741247795339884,
         0.003631342144629101, -0.00011271774634781165)
LN2 = float(np.log(2.0))

# which (layer, pair) slots run the DVE polynomial path (tuned for balance);
# ALT_SLOTS run the DVE path on odd tiles only (halved weights live in an
# extra packed block, ln2 added in-chain since shared biases can't alternate)
DVE_SLOTS = frozenset({(0, 3), (1, 3), (0, 7), (1, 7)})
ALT_SLOTS = ()
# slots whose polynomial chain runs on the GPSIMD (Pool) engine: the psum
# extract and square stay on DVE (GPSIMD cannot read psum), the SBUF-only
# Horner steps run on Pool
POOL_SLOTS = frozenset()  # real GPSIMD is far slower than the cost model


def _pin_act_tables(arch):
    """Restrict Exp/Ln to one table set so bacc emits a single table load."""
    tables = get_activation_tables(arch)
    for name, funcs in tables.items():
        if name != _ACT_SET:
            funcs.discard(F.Exp)
            funcs.discard(F.Ln)


def build(ntiles=NTILES, reps=1, nb=NB, ln_group=4, dve_slots=DVE_SLOTS,
          alt_slots=ALT_SLOTS, pool_slots=POOL_SLOTS, num_devices=NCORES):
    nc = bacc.Bacc("TRN2", target_bir_lowering=False, debug=False,
                   enable_asserts=True, num_devices=num_devices)
    _pin_act_tables(nc.m.arch)
    NB = nb
    G = ln_group

    # tiles batch in groups of `ob`: the final softplus runs once per batch on
    # an accumulator packing 4 tiles per [128, NB] block at partition offsets
    # 0/32/64/96 (dims in rows 32a..32a+15; rows 32a+16.. are memset garbage)
    ob = next(o for o in (8, 4, 2, 1) if ntiles % o == 0)
    ncolblk = max(1, ob // 4)

    xT = nc.dram_tensor("xT", [D + TE, BC], FP16, kind="ExternalInput")
    w1 = nc.dram_tensor("w1", [D + TE, NPAIR * 128], FP16, kind="ExternalInput")
    nw2 = NPAIR + sum(1 for s in alt_slots if s[0] == 1)
    w2 = nc.dram_tensor("w2", [128, nw2 * 128], FP16, kind="ExternalInput")
    w3 = nc.dram_tensor("w3", [128, NPAIR * 16], FP16, kind="ExternalInput")
    b1 = nc.dram_tensor("b1", [128, NPAIR], FP32, kind="ExternalInput")
    b2 = nc.dram_tensor("b2", [128, nw2], FP32, kind="ExternalInput")
    b3 = nc.dram_tensor("b3", [128, 1], FP32, kind="ExternalInput")
    # output row 32*(i%4)+d, col (i//4)*NB+j holds dim d of point i*NB+j
    nblk = (ntiles + 3) // 4
    out = nc.dram_tensor("out", [128, nblk * NB], FP16, kind="ExternalOutput")

    mm = nc.tensor.matmul

    def softplus_dve(vec, z, beta_ap, s_dst, u_dst, add_ln2=False, chain=None):
        """softplus(z+beta) = s + u (+ln2 folded downstream): s = (z+beta)/1,
        u = s^2*R(s^2).  s and u feed the next matmul as separate accumulating
        rhs streams, so no final tensor-tensor add is needed.  The psum
        extract and square always run on DVE; `chain` (default DVE) runs the
        SBUF-only Horner steps (nc.gpsimd offloads them to Pool)."""
        ch = chain or vec
        vec.tensor_scalar(s_dst, z, beta_ap, None, ALU.add)
        m = vec.pool.tile([128, NB], FP16, tag="m")
        vec.tensor_tensor(m, s_dst, s_dst, ALU.mult)
        r = vec.pool.tile([128, NB], FP16, tag="r")
        c = RPOLY
        ch.tensor_scalar(r, m, float(c[3]), float(c[2]), ALU.mult, ALU.add)
        for k in (1, 0):
            ch.tensor_tensor(r, r, m, ALU.mult)
            ch.tensor_scalar(r, r, float(c[k]), None, ALU.add)
        ch.tensor_tensor(u_dst, r, m, ALU.mult)
        if add_ln2:
            ch.tensor_scalar(u_dst, u_dst, LN2, None, ALU.add)

    with tile.TileContext(nc) as tc:
        with ExitStack() as ctx:
            wpool = ctx.enter_context(tc.tile_pool(name="w", bufs=1))
            xpool = ctx.enter_context(tc.tile_pool(name="x", bufs=2))
            hpool1 = ctx.enter_context(tc.tile_pool(name="h1", bufs=3))
            hpool2 = ctx.enter_context(tc.tile_pool(name="h2", bufs=2))
            apool = ctx.enter_context(tc.tile_pool(name="acc", bufs=1))
            opool = ctx.enter_context(tc.tile_pool(name="o", bufs=1))
            epool = ctx.enter_context(tc.tile_pool(name="e", bufs=3))
            e3pool = ctx.enter_context(tc.tile_pool(name="e3", bufs=1))
            vpool = ctx.enter_context(tc.tile_pool(name="v", bufs=2))
            zpool1 = ctx.enter_context(tc.tile_pool(name="z1", bufs=2, space="PSUM"))
            zpool2 = ctx.enter_context(tc.tile_pool(name="z2", bufs=2, space="PSUM"))

            class _Vec:
                pool = vpool
                tensor_scalar = nc.vector.tensor_scalar
                tensor_tensor = nc.vector.tensor_tensor
            vec = _Vec()

            w1sb = wpool.tile([D + TE, NPAIR * 128], FP16)
            w2sb = wpool.tile([128, nw2 * 128], FP16)
            w3sb = wpool.tile([128, NPAIR * 16], FP16)
            b1sb = wpool.tile([128, NPAIR], FP32)
            b2sb = wpool.tile([128, nw2], FP32)
            b3sb = wpool.tile([128, 1], FP32)
            nc.sync.dma_start(out=w1sb, in_=w1[:, :])
            nc.sync.dma_start(out=w2sb, in_=w2[:, :])
            nc.sync.dma_start(out=w3sb, in_=w3[:, :])
            nc.sync.dma_start(out=b1sb, in_=b1[:, :])
            nc.sync.dma_start(out=b2sb, in_=b2[:, :])
            nc.sync.dma_start(out=b3sb, in_=b3[:, :])

            ngroup = NPAIR // G

            def emit_l1(i):
                """DMA x tile, L1 matmuls + softplus. Returns h1src per group."""
                xt = xpool.tile([D + TE, NB], FP16)
                nc.sync.dma_start(out=xt, in_=xT[:, i * NB:(i + 1) * NB])
                h1srcs = []
                for g in range(ngroup):
                    pg = list(range(g * G, (g + 1) * G))
                    ea = [j for j, p in enumerate(pg)
                          if (0, p) not in dve_slots and (0, p) not in pool_slots]
                    e1g = epool.tile([128, G, NB], FP16, tag="e")
                    h1g = hpool1.tile([128, G, NB], FP16)
                    h1src = []
                    for j, p in enumerate(pg):
                        z1 = zpool1.tile([128, NB], FP32, tag="z1")
                        for q in range(NB // NMM):
                            s_ = slice(q * NMM, (q + 1) * NMM)
                            mm(z1[:, s_], w1sb[:, p * 128:(p + 1) * 128],
                               xt[:, s_], start=True, stop=True)
                        if (0, p) in dve_slots or (0, p) in pool_slots:
                            sd = hpool1.tile([128, NB], FP16, tag="s1")
                            ud = hpool1.tile([128, NB], FP16, tag="hd1")
                            h1src.append((sd, ud))
                            softplus_dve(vec, z1, b1sb[:, p:p + 1], sd, ud,
                                         chain=nc.gpsimd
                                         if (0, p) in pool_slots else None)
                        else:
                            h1src.append((h1g[:, j, :],))
                            nc.scalar.activation(e1g[:, j, :], z1, F.Exp,
                                                 bias=b1sb[:, p:p + 1])
                    if ea:
                        # contiguous runs only; assignment keeps A-slots contiguous
                        j0, j1 = ea[0], ea[-1] + 1
                        nc.scalar.activation(h1g[:, j0:j1, :], e1g[:, j0:j1, :],
                                             F.Ln, bias=1.0)
                    h1srcs.append(h1src)
                return h1srcs

            alt2 = {s: NPAIR + n for n, s in
                    enumerate(s for s in alt_slots if s[0] == 1)}

            def emit_l2(i, h1srcs):
                """L2 matmuls + softplus. Returns h2src per group."""
                odd = i % 2 == 1
                h2srcs = []
                for g in range(ngroup):
                    pg = list(range(g * G, (g + 1) * G))
                    h1src = h1srcs[g]
                    dve2 = [p for p in pg if (1, p) in dve_slots
                            or (1, p) in pool_slots or (odd and (1, p) in alt2)]
                    ea2 = [j for j, p in enumerate(pg) if p not in dve2]
                    e2g = epool.tile([128, G, NB], FP16, tag="e")
                    h2g = hpool2.tile([128, G, NB], FP16)
                    h2src = []
                    for j, p in enumerate(pg):
                        is_alt = odd and (1, p) in alt2
                        pb = alt2[(1, p)] if is_alt else p
                        z2 = zpool2.tile([128, NB], FP32, tag="z2")
                        srcs = h1src[j]
                        for q in range(NB // NMM):
                            s_ = slice(q * NMM, (q + 1) * NMM)
                            for si, src in enumerate(srcs):
                                mm(z2[:, s_], w2sb[:, pb * 128:(pb + 1) * 128],
                                   src[:, s_], start=(si == 0),
                                   stop=(si == len(srcs) - 1))
                        if p in dve2:
                            sd = hpool2.tile([128, NB], FP16, tag="s2")
                            ud = hpool2.tile([128, NB], FP16, tag="hd2")
                            h2src.append((sd, ud))
                            softplus_dve(vec, z2, b2sb[:, pb:pb + 1], sd, ud,
                                         add_ln2=is_alt,
                                         chain=nc.gpsimd
                                         if (1, p) in pool_slots else None)
                        else:
                            h2src.append((h2g[:, j, :],))
                            nc.scalar.activation(e2g[:, j, :], z2, F.Exp,
                                                 bias=b2sb[:, pb:pb + 1])
                    if ea2:
                        j0, j1 = ea2[0], ea2[-1] + 1
                        nc.scalar.activation(h2g[:, j0:j1, :], e2g[:, j0:j1, :],
                                             F.Ln, bias=1.0)
                    h2srcs.append(h2src)
                return h2srcs

            def emit_l3(i, h2srcs, acc):
                """L3 psum accumulation + fold into acc; final softplus per batch."""
                u, q3 = i % ob, i // ob
                arow = acc[32 * (u % 4):32 * (u % 4) + D,
                           (u // 4) * NB:(u // 4) * NB + NB]
                z3 = zpool2.tile([16, NB], FP32, tag="z2")
                for g in range(ngroup):
                    pg = list(range(g * G, (g + 1) * G))
                    h2src = h2srcs[g]
                    last_g = g == ngroup - 1
                    for q in range(NB // NMM):
                        s_ = slice(q * NMM, (q + 1) * NMM)
                        for j, p in enumerate(pg):
                            for si, src in enumerate(h2src[j]):
                                mm(z3[:, s_], w3sb[:, p * 16:(p + 1) * 16],
                                   src[:, s_],
                                   start=(g == 0 and j == 0 and si == 0),
                                   stop=(last_g and j == G - 1
                                         and si == len(h2src[j]) - 1))
                nc.vector.tensor_copy(arow, z3)
                if u == ob - 1:
                    e3 = e3pool.tile([128, ncolblk * NB], FP16)
                    nc.scalar.activation(e3, acc, F.Exp, bias=b3sb)
                    o3 = opool.tile([128, ncolblk * NB], FP16)
                    nc.scalar.activation(o3, e3, F.Ln, bias=1.0)
                    nc.sync.dma_start(
                        out=out[:, q3 * ncolblk * NB:(q3 + 1) * ncolblk * NB],
                        in_=o3)

            loop_cm = tc.For_i(0, reps, 1) if reps > 1 else nullcontext()
            with loop_cm:
                # software pipeline: L1 of tile i+1 is emitted before L3 of
                # tile i so the in-order PE queue never stalls on the DVE
                # chains feeding tile i's L3
                acc = None
                h1s = emit_l1(0)
                for i in range(ntiles):
                    if i % ob == 0:
                        acc = apool.tile([128, ncolblk * NB], FP32)
                        nc.vector.memset(acc, 0.0)
                    h2s = emit_l2(i, h1s)
                    if i + 1 < ntiles:
                        h1n = emit_l1(i + 1)
                    emit_l3(i, h2s, acc)
                    if i + 1 < ntiles:
                        h1s = h1n
    nc.compile()
    return nc


def _pack_inputs(t, y, W1, b1, W2, b2, W3, b3, dve_slots=DVE_SLOTS,
                 alt_slots=ALT_SLOTS, pool_slots=POOL_SLOTS):
    dve_slots = frozenset(dve_slots) | frozenset(pool_slots)
    """Host-side packing. Returns per-core input maps."""
    t = np.asarray(t, np.float32)
    y = np.asarray(y, np.float32)
    W1 = np.asarray(W1, np.float32)
    b1 = np.asarray(b1, np.float32)
    W2 = np.asarray(W2, np.float32)
    b2 = np.asarray(b2, np.float32)
    W3 = np.asarray(W3, np.float32)
    b3 = np.asarray(b3, np.float32)

    xT = np.empty((D + TE, B), np.float16)
    xT[:D] = y.T
    xT[D:D + TE] = t.T

    # ln2 constants dropped by the D-path fold forward into the next bias.
    l1_dve = np.array([(0, p) in dve_slots for p in range(NPAIR)])
    b2eff = b2 + np.where(
        np.repeat(l1_dve, 2)[:, None], LN2 * W2.sum(axis=1), 0.0)
    l2_dve = np.array([(1, p) in dve_slots for p in range(NPAIR)])
    b3eff = b3 + np.where(
        np.repeat(l2_dve, 2)[:, None], LN2 * W3.sum(axis=1), 0.0)

    alt2 = [s[1] for s in alt_slots if s[0] == 1]
    nw2 = NPAIR + len(alt2)
    w1p = np.zeros((D + TE, NPAIR * 128), np.float16)
    w2p = np.zeros((128, nw2 * 128), np.float16)
    w3p = np.zeros((128, NPAIR * 16), np.float16)
    b1p = np.zeros((128, NPAIR), np.float32)
    b2p = np.zeros((128, nw2), np.float32)
    b3p = np.zeros((128, 1), np.float32)
    for p in range(NPAIR):
        s1 = 0.5 if (0, p) in dve_slots else 1.0
        s2 = 0.5 if (1, p) in dve_slots else 1.0
        for a in range(2):
            d = 2 * p + a
            c = slice(p * 128 + 64 * a, p * 128 + 64 * a + 64)
            w1p[d, c] = s1 * W1[d, 0, :]
            w1p[D:D + TE, c] = s1 * W1[d, 1:1 + TE, :]
            w2p[64 * a:64 * a + 64, p * 128 + 64 * a:p * 128 + 64 * a + 64] = \
                s2 * W2[d]
            w3p[64 * a:64 * a + 64, p * 16 + d] = W3[d, :, 0]
            b1p[64 * a:64 * a + 64, p] = s1 * b1[d]
            b2p[64 * a:64 * a + 64, p] = s2 * b2eff[d]
            for a_ in range(4):
                b3p[32 * a_ + d, 0] = b3eff[d, 0]
    # alt blocks: halved W2/b2 for odd-tile DVE path (ln2 added in-chain)
    for n_, p in enumerate(alt2):
        pb = NPAIR + n_
        for a in range(2):
            d = 2 * p + a
            w2p[64 * a:64 * a + 64, pb * 128 + 64 * a:pb * 128 + 64 * a + 64] = \
                0.5 * W2[d]
            b2p[64 * a:64 * a + 64, pb] = 0.5 * b2eff[d]

    in_maps = []
    for c in range(NCORES):
        in_maps.append({
            "xT": np.ascontiguousarray(xT[:, c * BC:(c + 1) * BC]),
            "w1": w1p, "w2": w2p, "w3": w3p,
            "b1": b1p, "b2": b2p, "b3": b3p,
        })
    return in_maps


def _unpack_output(results):
    cores = []
    for c in range(NCORES):
        a = results[c]["out"]          # [128, nblk*NB]
        nblk = a.shape[1] // NB
        a = (a.reshape(4, 32, nblk, NB)[:, :D]
             .transpose(1, 2, 0, 3).reshape(D, nblk * 4 * NB))
        cores.append(a[:, :BC].T.astype(np.float32))
    return np.concatenate(cores, axis=0)


def make_runner(nc):
    """Build a reusable jitted SPMD callable for `nc` (axon PJRT path)."""
    import jax
    from jax.sharding import Mesh, PartitionSpec, NamedSharding
    from jax.experimental.shard_map import shard_map
    from concourse import bass2jax

    bass2jax.install_neuronx_cc_hook()
    partition_name = nc.partition_id_tensor.name if nc.partition_id_tensor else None
    in_names, out_names, out_avals = [], [], []
    for alloc in nc.m.functions[0].allocations:
        if not isinstance(alloc, mybir.MemoryLocationSet):
            continue
        name = alloc.memorylocations[0].name
        if alloc.kind == "ExternalInput":
            if name != partition_name:
                in_names.append(name)
        elif alloc.kind == "ExternalOutput":
            out_names.append(name)
            out_avals.append(jax.core.ShapedArray(tuple(alloc.tensor_shape),
                                                  mybir.dt.np(alloc.dtype)))
    all_in = in_names + out_names + ([partition_name] if partition_name else [])

    def _body(*args):
        operands = list(args)
        if partition_name is not None:
            operands.append(bass2jax.partition_id_tensor())
        outs = bass2jax._bass_exec_p.bind(
            *operands, out_avals=tuple(out_avals),
            in_names=tuple(all_in), out_names=tuple(out_names),
            lowering_input_output_aliases=(), sim_require_finite=True,
            sim_require_nnan=True, nc=nc)
        return tuple(outs)

    mesh = Mesh(np.asarray(jax.devices()[:NCORES]), ("core",))
    n = len(in_names) + len(out_names)
    sharded = jax.jit(shard_map(_body, mesh=mesh,
                                in_specs=(PartitionSpec("core"),) * n,
                                out_specs=(PartitionSpec("core"),) * len(out_names),
                                check_rep=False), keep_unused=True)
    shard0 = NamedSharding(mesh, PartitionSpec("core"))
    zeros = [jax.device_put(np.zeros((NCORES * a.shape[0], *a.shape[1:]), a.dtype),
                            shard0) for a in out_avals]

    def stage(in_maps):
        return [jax.device_put(
            np.concatenate([np.asarray(in_maps[c][nm]) for c in range(NCORES)], axis=0),
            shard0) for nm in in_names]

    def run_staged(dev_in):
        out_arrs = sharded(*dev_in, *zeros)
        jax.block_until_ready(out_arrs)
        return out_arrs

    def run(in_maps):
        out_arrs = run_staged(stage(in_maps))
        return [
            {name: np.asarray(out_arrs[i]).reshape(NCORES, *out_avals[i].shape)[c]
             for i, name in enumerate(out_names)}
            for c in range(NCORES)
        ]

    run.stage = stage
    run.run_staged = run_staged
    run.out_names = out_names
    run.out_avals = out_avals
    return run


_CACHED = {}


def _get_runner():
    if "runner" not in _CACHED:
        _CACHED["runner"] = make_runner(build())
    return _CACHED["runner"]


def kernel(t, y, W1, b1, W2, b2, W3, b3):
    run = _get_runner()
    in_maps = _pack_inputs(t, y, W1, b1, W2, b2, W3, b3)
    results = run(in_maps)
    return _unpack_output(results)
